# revision 20
# baseline (speedup 1.0000x reference)
"""Longformer layer stack (4 layers, sliding-window attention) on 8 TRN2 cores.

Sharding: data-parallel over batch (2) x sequence-parallel (4 blocks of 1024
tokens). Each core computes its sequence block; the banded attention needs a
W=256 token halo, exchanged between neighbors with a bf16 AllGather that is
overlapped with the next layer's halo-independent compute. All matmuls run in
bf16 (FWL weight loads, 1 cyc/col); residual/LN statistics stay fp32.
"""
import sys

sys.path.insert(0, '/opt/trn_rl_repo')

import numpy as np
import ml_dtypes

import concourse.bass as bass
import concourse.mybir as mybir
import concourse.tile as tile
from concourse import bacc
from concourse import bass_utils

F32 = mybir.dt.float32
F32R = mybir.dt.float32r
BF16 = mybir.dt.bfloat16
I32 = mybir.dt.int32
AF = mybir.ActivationFunctionType
ALU = mybir.AluOpType

NH = 12          # heads
DH = 64          # head dim
HD = 768         # model dim
FF = 3072        # ffn dim
W = 256          # one-sided window
L = 4            # layers
B = 2
S = 4096
EPS = 1e-12
N_CORES = 8
T_OWN = 1024     # tokens per core
T_EXT = 1536     # with halos
FT = 6           # model-dim 128-tiles
FFT = 24         # ffn-dim 128-tiles
NCH = 4          # local chunks of 256 queries
P = 128

# bias_all column layout: bq, bk, bo, b2, ls1, lb1, ls2, lb2 (6 each), b1 (24)
BC_BQ, BC_BK, BC_BO, BC_B2 = 0, 6, 12, 18
BC_LS1, BC_LB1, BC_LS2, BC_LB2, BC_B1 = 24, 30, 36, 42, 48


def _ln_t(nc, sb, ps, r_aps, out_aps, ones_r, ones_b, eps_sb, bias_sb,
          s_col, b_col):
    """LayerNorm over the partition (feature) axis of transposed tiles.

    r_aps: FT fp32r SBUF APs [128, 512] (input; clobbered).
    out_aps[ft] <- LN(r)*s + b.  s/b from bias_sb columns s_col/b_col.
    Stats PSUM comes from the shared pool `ps` tag "ops" (bufs=2 required).
    """
    ncols = 512
    sumx = ps.tile([1, ncols], F32, tag="ops", name="lnsum", bufs=2)
    sumsq = ps.tile([1, ncols], F32, tag="ops", name="lnsumsq", bufs=2)
    for ft in range(FT):
        sq = sb.tile([P, ncols], BF16, tag="lnsq", name="lnsq", bufs=1)
        nc.scalar.activation(sq[:], r_aps[ft], AF.Square)
        nc.tensor.matmul(sumx[0:1, :], ones_r[:, 0:1], r_aps[ft],
                         start=(ft == 0), stop=(ft == FT - 1))
        nc.tensor.matmul(sumsq[0:1, :], ones_b[:, 0:1], sq[:],
                         start=(ft == 0), stop=(ft == FT - 1))
    mu = sb.tile([1, ncols], F32, tag="lnmu", name="lnmu")
    nc.scalar.activation(mu[:], sumx[0:1, :], AF.Identity, scale=1.0 / HD)
    musq = sb.tile([1, ncols], F32, tag="lnmusq", name="lnmusq")
    nc.vector.tensor_tensor(musq[:], mu[:], mu[:], op=ALU.mult)
    var = sb.tile([1, ncols], F32, tag="lnvar", name="lnvar")
    nc.vector.scalar_tensor_tensor(out=var[:], in0=sumsq[0:1, :],
                                   scalar=1.0 / HD, in1=musq[:],
                                   op0=ALU.mult, op1=ALU.subtract)
    rstd = sb.tile([1, ncols], F32, tag="lnrstd", name="lnrstd")
    nc.scalar.activation(rstd[:], var[:], AF.Abs_reciprocal_sqrt,
                         bias=eps_sb[0:1, :])
    mu_b = sb.tile([P, ncols], F32, tag="lnmub", name="lnmub")
    nc.gpsimd.partition_broadcast(mu_b[:], mu[:], channels=P)
    rstd_b = sb.tile([P, ncols], F32, tag="lnrstdb", name="lnrstdb")
    nc.gpsimd.partition_broadcast(rstd_b[:], rstd[:], channels=P)
    for ft in range(FT):
        nc.vector.tensor_tensor(r_aps[ft], r_aps[ft], mu_b[:], op=ALU.subtract)
        nc.vector.tensor_tensor(r_aps[ft], r_aps[ft], rstd_b[:], op=ALU.mult)
        nc.scalar.activation(out_aps[ft], r_aps[ft], AF.Identity,
                             scale=bias_sb[:, s_col + ft:s_col + ft + 1],
                             bias=bias_sb[:, b_col + ft:b_col + ft + 1])


def build_nc(n_layers=L):
    nc = bacc.Bacc("TRN2", target_bir_lowering=False, debug=False,
                   num_devices=N_CORES)
    dt_ = nc.dram_tensor
    t = {}
    t["emb"] = dt_("emb_word", [32000, HD], F32, kind="ExternalInput").ap()
    t["ids"] = dt_("ids", [P, 12], I32, kind="ExternalInput").ap()
    t["pos"] = dt_("pos", [T_EXT, HD], F32, kind="ExternalInput").ap()
    t["eln_s"] = dt_("eln_s", [HD], F32, kind="ExternalInput").ap()
    t["eln_b"] = dt_("eln_b", [HD], F32, kind="ExternalInput").ap()
    t["wqk"] = dt_("wqk", [L, P, 12 * HD], BF16, kind="ExternalInput").ap()
    t["wv"] = dt_("wv", [L, P, FT * HD], BF16, kind="ExternalInput").ap()
    t["wo"] = dt_("wo", [L, P, FT * HD], BF16, kind="ExternalInput").ap()
    t["w1"] = dt_("w1", [L, 8, P, 3 * HD], BF16, kind="ExternalInput").ap()
    t["w2"] = dt_("w2", [L, FT, P, FFT * P], BF16, kind="ExternalInput").ap()
    t["bias"] = dt_("bias", [L, P, 72], F32, kind="ExternalInput").ap()
    t["sel"] = dt_("sel", [NH, FT * P], BF16, kind="ExternalInput").ap()
    t["ml"] = dt_("ml", [NCH, P, 512], BF16, kind="ExternalInput").ap()
    t["mr"] = dt_("mr", [NCH, P, 512], BF16, kind="ExternalInput").ap()
    t["halo_ids"] = dt_("halo_ids", [P, 12], I32, kind="ExternalInput").ap()
    t["out"] = dt_("out", [FT, P, T_OWN], F32, kind="ExternalOutput").ap()

    with tile.TileContext(nc) as tc:
        _build_body(nc, tc, n_layers, t)
    nc.compile()
    return nc


def _build_body(nc, tc, n_layers, t):
    from contextlib import ExitStack
    with ExitStack() as ctx:
        persist = ctx.enter_context(tc.tile_pool(name="persist", bufs=1))
        # residual stream, transposed, bf16, with halos: x[ft] = [128, T_EXT]
        x = [persist.tile([P, T_EXT], BF16, tag=f"x{ft}", name=f"x{ft}")
             for ft in range(FT)]
        ml_sb = [persist.tile([P, 512], BF16, tag=f"ml{c}", name=f"ml{c}")
                 for c in range(NCH)]
        mr_sb = [persist.tile([P, 512], BF16, tag=f"mr{c}", name=f"mr{c}")
                 for c in range(NCH)]
        for c in range(NCH):
            nc.sync.dma_start(ml_sb[c][:], t["ml"][c])
            nc.sync.dma_start(mr_sb[c][:], t["mr"][c])
        sel_sb = persist.tile([NH, FT * P], BF16, tag="sel", name="sel")
        nc.sync.dma_start(sel_sb[:], t["sel"][:])
        ones_f = persist.tile([P, 1], F32, tag="ones_f", name="ones_f")
        nc.vector.memset(ones_f[:], 1.0)
        ones_r = persist.tile([P, 1], F32R, tag="ones_r", name="ones_r")
        nc.scalar.activation(ones_r[:], ones_f[:], AF.Identity)
        ones_b = persist.tile([P, 1], BF16, tag="ones_b", name="ones_b")
        nc.scalar.activation(ones_b[:], ones_f[:], AF.Identity)
        from concourse.masks import make_identity
        ident = persist.tile([P, P], F32, tag="ident", name="ident")
        make_identity(nc, ident[:])
        hid_sb = persist.tile([P, 12], I32, tag="hid", name="hid")
        nc.sync.dma_start(hid_sb[:], t["halo_ids"][:])
        eps_sb = persist.tile([P, 1], F32, tag="eps", name="eps")
        nc.vector.memset(eps_sb[:], EPS)

        # ---- embedding + LN -> x^T (bf16, full ext incl. halos) ----
        with tc.tile_pool(name="emb_sb", bufs=1) as esb, \
             tc.tile_pool(name="emb_sb2", bufs=2) as esb2, \
             tc.tile_pool(name="emb_ps", bufs=2, space="PSUM") as eps_ps:
            ids_sb = esb.tile([P, 12], I32, tag="ids", name="ids")
            nc.sync.dma_start(ids_sb[:], t["ids"][:])
            s_bc = esb.tile([P, HD], F32, tag="sbc", name="sbc")
            nc.sync.dma_start(s_bc[:], bass.AP(
                tensor=t["eln_s"].tensor, offset=0, ap=[[0, P], [1, HD]]))
            b_bc = esb.tile([P, HD], F32, tag="bbc", name="bbc")
            nc.sync.dma_start(b_bc[:], bass.AP(
                tensor=t["eln_b"].tensor, offset=0, ap=[[0, P], [1, HD]]))
            e = [esb.tile([P, HD], F32, tag=f"e{tt}", name=f"e{tt}")
                 for tt in range(12)]
            for tt in range(12):
                nc.gpsimd.indirect_dma_start(
                    out=e[tt][:], out_offset=None, in_=t["emb"][:],
                    in_offset=bass.IndirectOffsetOnAxis(
                        ap=ids_sb[:, tt:tt + 1], axis=0))
                p_sb = esb2.tile([P, HD], F32, tag="pos", name="pos")
                nc.sync.dma_start(p_sb[:], t["pos"][tt * P:(tt + 1) * P, :])
                nc.vector.tensor_tensor(e[tt][:], e[tt][:], p_sb[:], op=ALU.add)
                stats = esb2.tile([P, 3, nc.vector.BN_STATS_DIM], F32,
                                  tag="bst", name="bst")
                er = e[tt][:].rearrange("p (g d) -> p g d", g=3)
                for g in range(3):
                    nc.vector.bn_stats(stats[:, g, :], er[:, g, :])
                mv = esb2.tile([P, nc.vector.BN_AGGR_DIM], F32, tag="bag",
                               name="bag")
                nc.vector.bn_aggr(mv[:], stats[:])
                sd = esb2.tile([P, 1], F32, tag="bsd", name="bsd")
                nc.scalar.activation(sd[:], mv[:, 1:2], AF.Sqrt, bias=eps_sb[:])
                rstd = esb2.tile([P, 1], F32, tag="brstd", name="brstd")
                nc.vector.reciprocal_approx_fast(rstd[:], sd[:])
                nc.vector.tensor_scalar(out=e[tt][:], in0=e[tt][:],
                                        scalar1=mv[:, 0:1], scalar2=rstd[:],
                                        op0=ALU.subtract, op1=ALU.mult)
                nc.vector.tensor_tensor(e[tt][:], e[tt][:], s_bc[:],
                                        op=ALU.mult)
                nc.vector.tensor_tensor(e[tt][:], e[tt][:], b_bc[:],
                                        op=ALU.add)
            for ft in range(FT):
                tr = eps_ps.tile([P, T_EXT], F32, tag="tr", name="tr")
                for tt in range(12):
                    nc.tensor.transpose(tr[:, tt * P:(tt + 1) * P],
                                        e[tt][:, ft * P:(ft + 1) * P],
                                        ident[:])
                nc.scalar.activation(x[ft][:], tr[:], AF.Identity)

        scatter_fn = None
        for l in range(n_layers):
            scatter_fn = _layer(nc, tc, t, l, x, ml_sb, mr_sb, ones_r, ones_b,
                                eps_sb, hid_sb, sel_sb, scatter_fn,
                                exchange=(l < n_layers - 1),
                                last=(l == n_layers - 1))


def _layer(nc, tc, t, l, x, ml_sb, mr_sb, ones_r, ones_b, eps_sb, hid_sb,
           sel_sb, consume_halo, exchange, last):
    from contextlib import ExitStack
    with ExitStack() as ctx:
        lsb = ctx.enter_context(tc.tile_pool(name=f"lsb{l}", bufs=1))
        lps = ctx.enter_context(tc.tile_pool(name=f"lps{l}", bufs=1,
                                             space="PSUM"))

        bias_sb = lsb.tile([P, 72], F32, tag="bias", name="bias")
        nc.sync.dma_start(bias_sb[:], t["bias"][l])
        wo_sb = lsb.tile([P, FT * HD], BF16, tag="wo", name="wo")
        nc.sync.dma_start(wo_sb[:], t["wo"][l])

        qT = [lsb.tile([P, T_OWN], BF16, tag=f"qT{i}", name=f"qT{i}")
              for i in range(FT)]
        kT = [lsb.tile([P, T_EXT], BF16, tag=f"kT{i}", name=f"kT{i}")
              for i in range(FT)]
        v = [lsb.tile([P, NH, 65], BF16, tag=f"v{i}", name=f"v{i}")
             for i in range(12)]
        o = [lsb.tile([P, T_OWN], BF16, tag=f"o{i}", name=f"o{i}")
             for i in range(FT)]
        r1 = [lsb.tile([P, T_OWN], F32R, tag=f"r1_{i}", name=f"r1_{i}")
              for i in range(FT)]
        den = lsb.tile([NH, T_OWN], F32, tag="den", name="den")

        # y (LN1 out) aliases kT's first T_OWN columns (kT dead by then)
        y = [kT[i][:, 0:T_OWN] for i in range(FT)]
        # r2 aliases r1 (r1 dead after LN1 consumed it)
        r2 = r1

        paw = tc.alloc_tile_pool(name=f"paw{l}", bufs=1)
        wqk = paw.tile([P, 12 * HD], BF16, tag="wqk", name="wqk")
        nc.sync.dma_start(wqk[:], t["wqk"][l])
        wv_sb = paw.tile([P, FT * HD], BF16, tag="wv", name="wv")
        nc.sync.dma_start(wv_sb[:], t["wv"][l])

        def proj_qT(mt, h2):
            ws = wqk[:, mt * HD:(mt + 1) * HD]
            ps = lps.tile([P, 512], F32, tag="pp", name="pq", bufs=3)
            for kt in range(FT):
                nc.tensor.matmul(
                    ps[:], ws[:, kt * P:(kt + 1) * P],
                    x[kt][:, W + h2 * 512:W + (h2 + 1) * 512],
                    start=(kt == 0), stop=(kt == FT - 1))
            nc.scalar.activation(qT[mt][:, h2 * 512:(h2 + 1) * 512], ps[:],
                                 AF.Identity,
                                 bias=bias_sb[:, BC_BQ + mt:BC_BQ + mt + 1])

        def proj_kT(mt, c0, n):
            ws = wqk[:, (6 + mt) * HD:(7 + mt) * HD]
            ps = lps.tile([P, 512], F32, tag="pp", name="pk", bufs=3)
            for kt in range(FT):
                nc.tensor.matmul(ps[:, 0:n], ws[:, kt * P:(kt + 1) * P],
                                 x[kt][:, c0:c0 + n],
                                 start=(kt == 0), stop=(kt == FT - 1))
            nc.scalar.activation(kT[mt][:, c0:c0 + n], ps[:, 0:n],
                                 AF.Identity,
                                 bias=bias_sb[:, BC_BK + mt:BC_BK + mt + 1])

        def proj_v(tt, hf):
            ps = lps.tile([P, 512], F32, tag="pp", name="pv", bufs=3)
            for kt in range(FT):
                nc.tensor.matmul(
                    ps[:, 0:384], x[kt][:, tt * P:(tt + 1) * P],
                    wv_sb[:, kt * HD + hf * 384:kt * HD + (hf + 1) * 384],
                    start=(kt == 0), stop=(kt == FT - 1))
            nc.scalar.activation(
                v[tt][:, hf * 6:(hf + 1) * 6, 0:64],
                ps[:, 0:384].rearrange("p (h d) -> p h d", h=6), AF.Identity)
            if hf == 1:
                nc.vector.memset(v[tt][:, :, 64:65], 1.0)

        def attn_head(c, h):
            ft, po = h // 2, (h % 2) * 64
            sps = lps.tile([P, 6 * W], F32, tag="sps", name="sps", bufs=1)
            for w in range(6):
                nc.tensor.matmul(
                    sps[:, w * W:(w + 1) * W],
                    kT[ft][po:po + 64, c * W + w * P:c * W + (w + 1) * P],
                    qT[ft][po:po + 64, c * W:(c + 1) * W],
                    start=True, stop=True)
            ex = lsb.tile([P, 6 * W], BF16, tag="ex", name="ex", bufs=2)
            nc.scalar.activation(ex[:], sps[:], AF.Exp)
            nc.vector.tensor_tensor(ex[:, 0:512], ex[:, 0:512], ml_sb[c][:],
                                    op=ALU.mult)
            nc.vector.tensor_tensor(ex[:, 1024:1536], ex[:, 1024:1536],
                                    mr_sb[c][:], op=ALU.mult)
            ops = lps.tile([P, W], F32, tag="ops", name="ops", bufs=2)
            for w in range(6):
                nc.tensor.matmul(ops[0:65, :], v[c * 2 + w][:, h, :],
                                 ex[:, w * W:(w + 1) * W],
                                 start=(w == 0), stop=(w == 5))
            sden = lsb.tile([1, W], F32, tag="sden", name="sden", bufs=2)
            nc.vector.tensor_copy(sden[:], ops[64:65, :])
            nc.sync.dma_start(den[h:h + 1, c * W:(c + 1) * W], sden[:])
            nc.vector.tensor_copy(o[ft][po:po + 64, c * W:(c + 1) * W],
                                  ops[0:64, :])

        def attn_chunk(c, fillers):
            for h in range(NH):
                attn_head(c, h)
                if fillers:
                    fillers.pop(0)()
            normalize_chunk(c)

        def normalize_chunk(c):
            cs = slice(c * W, (c + 1) * W)
            rec = lsb.tile([NH, W], F32, tag="rec", name="rec", bufs=2)
            nc.vector.reciprocal(rec[:], den[0:NH, cs])
            recb = lsb.tile([NH, W], BF16, tag="recb", name="recb", bufs=2)
            nc.scalar.activation(recb[:], rec[:], AF.Identity)
            for ft in range(FT):
                bc = lps.tile([P, W], F32, tag="ops", name="bc", bufs=2)
                nc.tensor.matmul(bc[:], sel_sb[0:NH, ft * P:(ft + 1) * P],
                                 recb[:], start=True, stop=True)
                nc.vector.tensor_tensor(o[ft][:, cs], o[ft][:, cs], bc[:],
                                        op=ALU.mult)

        def oproj_half(h2):
            cs = slice(h2 * 512, (h2 + 1) * 512)
            for mt in range(FT):
                ps = lps.tile([P, 512], F32, tag="pp", name="po", bufs=3)
                for kt in range(FT):
                    nc.tensor.matmul(ps[:],
                                     wo_sb[:, mt * HD + kt * P:
                                           mt * HD + (kt + 1) * P],
                                     o[kt][:, cs],
                                     start=(kt == 0), stop=(kt == FT - 1))
                nc.vector.scalar_tensor_tensor(
                    out=r1[mt][:, cs], in0=ps[:],
                    scalar=bias_sb[:, BC_BO + mt:BC_BO + mt + 1],
                    in1=x[mt][:, W + h2 * 512:W + (h2 + 1) * 512],
                    op0=ALU.add, op1=ALU.add)

        # ================= emission =================
        # A (halo-independent): qT h0, kT own, v own tt2..7
        for mt in range(FT):
            proj_qT(mt, 0)
        for mt in range(FT):
            proj_kT(mt, W, 512)
            proj_kT(mt, W + 512, 512)
        for tt in range(2, 8):
            proj_v(tt, 0)
            proj_v(tt, 1)

        # B(c1) with fillers: qT h1 (6), v tt8,9 (4)
        f1 = [lambda mt=mt: proj_qT(mt, 1) for mt in range(FT)]
        f1 += [lambda tt=tt, hf=hf: proj_v(tt, hf)
               for tt in (8, 9) for hf in (0, 1)]
        attn_chunk(1, f1)

        # halo arrives: scatter previous layer's collective into x halo cols
        if consume_halo is not None:
            consume_halo()

        # B(c2) with fillers: kT halo-left (6), v tt0,1 (4)
        f2 = [lambda mt=mt: proj_kT(mt, 0, W) for mt in range(FT)]
        f2 += [lambda tt=tt, hf=hf: proj_v(tt, hf)
               for tt in (0, 1) for hf in (0, 1)]
        attn_chunk(2, f2)

        # B(c0) with fillers: kT halo-right (6), v tt10,11 (4)
        f0 = [lambda mt=mt: proj_kT(mt, T_OWN + W, W) for mt in range(FT)]
        f0 += [lambda tt=tt, hf=hf: proj_v(tt, hf)
               for tt in (10, 11) for hf in (0, 1)]
        attn_chunk(0, f0)
        paw.release()

        # O-proj + residual for half 0, then LN1 half 0
        oproj_half(0)
        _ln_t(nc, lsb, lps, [r1[ft][:, 0:512] for ft in range(FT)],
              [y[ft][:, 0:512] for ft in range(FT)],
              ones_r, ones_b, eps_sb, bias_sb, BC_LS1, BC_LB1)

        # FFN pools (opened after paw released)
        pd = ctx.enter_context(tc.tile_pool(name=f"pd{l}", bufs=1))

        def ffn_pass1(h2, ms0, n_ms, f_list):
            cs = slice(h2 * 512, (h2 + 1) * 512)
            for ms in range(ms0, ms0 + n_ms):
                if ms % 3 == 0:
                    w1c = pd.tile([P, 3 * HD], BF16, tag="w1c", name="w1c",
                                  bufs=2)
                    nc.sync.dma_start(w1c[:], t["w1"][l, ms // 3])
                    ffn_pass1.w1c = w1c
                ws = ffn_pass1.w1c[:, (ms % 3) * HD:(ms % 3 + 1) * HD]
                fp = lps.tile([P, 512], F32, tag="pp", name="fp", bufs=3)
                for kt in range(FT):
                    nc.tensor.matmul(fp[:], ws[:, kt * P:(kt + 1) * P],
                                     y[kt][:, cs],
                                     start=(kt == 0), stop=(kt == FT - 1))
                fsb = pd.tile([P, 512], BF16, tag="fc", name="fsb", bufs=24)
                nc.scalar.activation(fsb[:], fp[:], AF.Gelu,
                                     bias=bias_sb[:, BC_B1 + ms:BC_B1 + ms + 1])
                f_list.append(fsb)

        def ffn_pass2(h2, f_list):
            cs = slice(h2 * 512, (h2 + 1) * 512)
            for mt in range(FT):
                w2p = pd.tile([P, FFT * P], BF16, tag="w2p", name="w2p",
                              bufs=2)
                nc.sync.dma_start(w2p[:], t["w2"][l, mt])
                zp = lps.tile([P, 512], F32, tag="ops", name="zp", bufs=2)
                for ms in range(FFT):
                    nc.tensor.matmul(zp[:], w2p[:, ms * P:(ms + 1) * P],
                                     f_list[ms][:],
                                     start=(ms == 0), stop=(ms == FFT - 1))
                nc.vector.scalar_tensor_tensor(
                    out=r2[mt][:, cs], in0=zp[:],
                    scalar=bias_sb[:, BC_B2 + mt:BC_B2 + mt + 1],
                    in1=y[mt][:, cs], op0=ALU.add, op1=ALU.add)

        def ln2_half(h2):
            csx = slice(W + h2 * 512, W + (h2 + 1) * 512)
            cs = slice(h2 * 512, (h2 + 1) * 512)
            if last:
                xo = pd.tile([P, FT, 512], F32, tag="xout", name="xo", bufs=1)
                _ln_t(nc, lsb, lps, [r2[ft][:, cs] for ft in range(FT)],
                      [xo[:, ft, :] for ft in range(FT)],
                      ones_r, ones_b, eps_sb, bias_sb, BC_LS2, BC_LB2)
                for ft in range(FT):
                    nc.gpsimd.dma_start(
                        t["out"][ft, :, h2 * 512:(h2 + 1) * 512],
                        xo[:, ft, :])
            else:
                _ln_t(nc, lsb, lps, [r2[ft][:, cs] for ft in range(FT)],
                      [x[ft][:, csx] for ft in range(FT)],
                      ones_r, ones_b, eps_sb, bias_sb, BC_LS2, BC_LB2)

        # B(c3) interleaved with FFN(h0) pass1 in coarse blocks (limits
        # scalar activation-table thrash between Exp and Gelu)
        f_h0 = []
        for h in range(6):
            attn_head(3, h)
        ffn_pass1(0, 0, 12, f_h0)
        for h in range(6, NH):
            attn_head(3, h)
        normalize_chunk(3)
        ffn_pass1(0, 12, 12, f_h0)
        ffn_pass2(0, f_h0)
        ln2_half(0)

        if exchange:
            edram = ctx.enter_context(
                tc.tile_pool(name=f"pe{l}", bufs=1, space="DRAM"))
            b_in = edram.tile([2, FT, P, W], BF16, tag="bin", name="bin")
            b_out = edram.tile([4 * 2 * FT * P, W], BF16, tag="bout",
                               name="bout")
            for ft in range(FT):
                nc.sync.dma_start(b_in[0, ft], x[ft][:, W:2 * W])

        # O-proj half 1 + LN1 half 1 + FFN(h1) + LN2 half 1
        oproj_half(1)
        _ln_t(nc, lsb, lps, [r1[ft][:, 512:1024] for ft in range(FT)],
              [y[ft][:, 512:1024] for ft in range(FT)],
              ones_r, ones_b, eps_sb, bias_sb, BC_LS1, BC_LB1)
        f_h1 = []
        ffn_pass1(1, 0, FFT, f_h1)
        ffn_pass2(1, f_h1)
        ln2_half(1)

        if not exchange:
            return None

        for ft in range(FT):
            nc.sync.dma_start(b_in[1, ft], x[ft][:, T_OWN:T_OWN + W])
        nc.gpsimd.collective_compute(
            "AllGather", ALU.bypass,
            replica_groups=[[0, 1, 2, 3], [4, 5, 6, 7]],
            ins=[b_in[:].opt()], outs=[b_out[:].opt()])

        def scatter():
            for side in range(2):
                for ft in range(FT):
                    dst = (x[ft][:, 0:W] if side == 0
                           else x[ft][:, T_OWN + W:T_EXT])
                    nc.gpsimd.indirect_dma_start(
                        out=dst, out_offset=None, in_=b_out[:],
                        in_offset=bass.IndirectOffsetOnAxis(
                            ap=hid_sb[:, side * FT + ft:side * FT + ft + 1],
                            axis=0))
        return scatter


# ---------------- host side ----------------

def _blocked(w, n_k, n_m):
    """[n_k*128, n_m*128] -> [n_m, 128, n_k, 128] (lhsT strips by out-tile)."""
    return np.ascontiguousarray(
        w.reshape(n_k, P, n_m, P).transpose(2, 1, 0, 3))


def _bias_lay(b, n):
    return np.ascontiguousarray(b.reshape(n, P).T)


def prepare(inputs):
    """Build per-core in_maps from full inputs."""
    bf = ml_dtypes.bfloat16
    ids_full = np.asarray(inputs["input_ids"]).astype(np.int32)
    am = np.asarray(inputs["attention_mask"]).astype(np.int32)
    emb_word = np.asarray(inputs["emb_word"], dtype=np.float32)
    emb_pos = np.asarray(inputs["emb_pos"], dtype=np.float32)
    Wq = np.asarray(inputs["Wq"], np.float32) / np.sqrt(DH)
    bq = np.asarray(inputs["bq"], np.float32) / np.sqrt(DH)
    Wk = np.asarray(inputs["Wk"], np.float32)
    bk = np.asarray(inputs["bk"], np.float32)
    Wv = np.asarray(inputs["Wv"], np.float32)
    bv = np.asarray(inputs["bv"], np.float32)
    Wo = np.asarray(inputs["Wo"], np.float32)
    bo = np.asarray(inputs["bo"], np.float32)
    W1 = np.asarray(inputs["W1"], np.float32)
    b1 = np.asarray(inputs["b1"], np.float32)
    W2 = np.asarray(inputs["W2"], np.float32)
    b2 = np.asarray(inputs["b2"], np.float32)
    assert np.all(am == 1), "general attention_mask needs mid-tile masks too"

    wqk = np.empty((L, P, 12 * HD), np.float32)
    wv_flat = np.empty((L, P, FT * HD), np.float32)
    wo_flat = np.empty((L, P, FT * HD), np.float32)
    w1c = np.empty((L, 8, P, 3 * HD), np.float32)
    w2p = np.empty((L, FT, P, FFT * P), np.float32)
    for i in range(L):
        bq_s = _blocked(Wq[i], FT, FT)   # [mt, P, kt*P]
        bk_s = _blocked(Wk[i], FT, FT)
        bo_s = _blocked(Wo[i], FT, FT)
        for mt in range(FT):
            wqk[i, :, mt * HD:(mt + 1) * HD] = bq_s[mt].reshape(P, HD)
            wqk[i, :, (6 + mt) * HD:(7 + mt) * HD] = bk_s[mt].reshape(P, HD)
            wo_flat[i, :, mt * HD:(mt + 1) * HD] = bo_s[mt].reshape(P, HD)
        wv_flat[i] = Wv[i].reshape(FT, P, HD).transpose(1, 0, 2).reshape(
            P, FT * HD)
        b1_s = _blocked(W1[i], FT, FFT)  # [FFT, P, HD]
        w1c[i] = b1_s.reshape(8, 3, P, HD).transpose(0, 2, 1, 3).reshape(
            8, P, 3 * HD)
        w2p[i] = W2[i].reshape(FFT, P, FT, P).transpose(2, 1, 0, 3).reshape(
            FT, P, FFT * P)

    bias_all = np.zeros((L, P, 72), np.float32)
    for i in range(L):
        bias_all[i, :, BC_BQ:BC_BQ + 6] = _bias_lay(bq[i], FT)
        bias_all[i, :, BC_BK:BC_BK + 6] = _bias_lay(bk[i], FT)
        bias_all[i, :, BC_BO:BC_BO + 6] = _bias_lay(bv[i] @ Wo[i] + bo[i], FT)
        bias_all[i, :, BC_B2:BC_B2 + 6] = _bias_lay(b2[i], FT)
        bias_all[i, :, BC_LS1:BC_LS1 + 6] = _bias_lay(
            np.asarray(inputs["ln1_s"], np.float32)[i], FT)
        bias_all[i, :, BC_LB1:BC_LB1 + 6] = _bias_lay(
            np.asarray(inputs["ln1_b"], np.float32)[i], FT)
        bias_all[i, :, BC_LS2:BC_LS2 + 6] = _bias_lay(
            np.asarray(inputs["ln2_s"], np.float32)[i], FT)
        bias_all[i, :, BC_LB2:BC_LB2 + 6] = _bias_lay(
            np.asarray(inputs["ln2_b"], np.float32)[i], FT)
        bias_all[i, :, BC_B1:BC_B1 + 24] = _bias_lay(b1[i], FFT)

    sel = np.zeros((NH, FT * P), np.float32)
    for ft in range(FT):
        for p in range(P):
            sel[2 * ft + (p >= 64), ft * P + p] = 1.0

    shared = {
        "emb_word": emb_word,
        "eln_s": np.asarray(inputs["emb_ln_s"], np.float32),
        "eln_b": np.asarray(inputs["emb_ln_b"], np.float32),
        "wqk": wqk.astype(bf),
        "wv": wv_flat.astype(bf),
        "wo": wo_flat.astype(bf),
        "w1": w1c.astype(bf),
        "w2": w2p.astype(bf),
        "bias": bias_all,
        "sel": sel.astype(bf),
    }

    in_maps = []
    i_idx = np.arange(W)
    for core in range(N_CORES):
        b, sb = core // 4, core % 4
        s0 = sb * T_OWN
        ext_pos = np.clip(np.arange(s0 - W, s0 + T_OWN + W), 0, S - 1)
        m = dict(shared)
        m["ids"] = np.ascontiguousarray(
            ids_full[b, ext_pos].reshape(12, P).T)
        m["pos"] = np.ascontiguousarray(emb_pos[ext_pos])
        # masks: global chunk gc, window key j in [0,768), query i in [0,256):
        #   key_abs = gc*W - W + j ; allowed = |j - W - i| <= W
        #             & 0 <= key_abs < S & attention_mask[b, key_abs]
        mlm = np.zeros((NCH, P, 512), np.float32)
        mrm = np.zeros((NCH, P, 512), np.float32)
        for c in range(NCH):
            gc = sb * NCH + c
            for kt2 in range(2):
                for mm_, j0 in ((mlm, 0), (mrm, 512)):
                    j = j0 + kt2 * P + np.arange(P)[:, None]
                    key_abs = gc * W - W + j
                    ok = (np.abs(j - W - i_idx[None, :]) <= W)
                    ok &= (key_abs >= 0) & (key_abs < S)
                    ok &= am[b, np.clip(key_abs, 0, S - 1)] > 0
                    mm_[c, :, kt2 * W:(kt2 + 1) * W] = ok
        m["ml"] = mlm.astype(bf)
        m["mr"] = mrm.astype(bf)
        # halo row ids into the gathered [4, 2, FT, 128, W] row table
        hid = np.zeros((2, FT, P), np.int64)
        for side in range(2):
            nb = sb - 1 if side == 0 else sb + 1
            if 0 <= nb <= 3:
                osd = 1 - side  # left halo <- neighbor's right block
                for ft in range(FT):
                    hid[side, ft] = ((nb * 2 + osd) * FT + ft) * P \
                        + np.arange(P)
            else:
                for ft in range(FT):
                    hid[side, ft] = ((sb * 2 + side) * FT + ft) * P \
                        + np.arange(P)
        m["halo_ids"] = np.ascontiguousarray(
            hid.reshape(12, P).T.astype(np.int32))
        in_maps.append(m)
    return in_maps


_NC_CACHE = {}


def get_nc(n_layers=L):
    if n_layers not in _NC_CACHE:
        _NC_CACHE[n_layers] = build_nc(n_layers)
    return _NC_CACHE[n_layers]


def run(inputs, n_layers=L, trace=False):
    nc = get_nc(n_layers)
    in_maps = prepare(inputs)
    res = bass_utils.run_bass_kernel_spmd(
        nc, in_maps, core_ids=list(range(N_CORES)), trace=trace)
    outs = np.empty((B, S, HD), np.float32)
    for core in range(N_CORES):
        b, sb = core // 4, core % 4
        ot = res.results[core]["out"]  # [FT, 128, T_OWN]
        outs[b, sb * T_OWN:(sb + 1) * T_OWN] = ot.reshape(HD, T_OWN).T
    return outs, res


def kernel(**inputs) -> np.ndarray:
    out, _ = run(inputs)
    return out


# revision 22
# speedup vs baseline: 1.0450x; 1.0450x over previous
"""Longformer layer stack (4 layers, sliding-window attention) on 8 TRN2 cores.

Sharding: data-parallel over batch (2) x sequence-parallel (4 blocks of 1024
tokens). Each core computes its sequence block; the banded attention needs a
W=256 token halo, exchanged between neighbors with a bf16 AllGather that is
overlapped with the next layer's halo-independent compute. All matmuls run in
bf16 (FWL weight loads, 1 cyc/col); residual/LN statistics stay fp32.
"""
import sys

sys.path.insert(0, '/opt/trn_rl_repo')

import numpy as np
import ml_dtypes

import concourse.bass as bass
import concourse.mybir as mybir
import concourse.tile as tile
from concourse import bacc
from concourse import bass_utils

F32 = mybir.dt.float32
F32R = mybir.dt.float32r
BF16 = mybir.dt.bfloat16
I32 = mybir.dt.int32
AF = mybir.ActivationFunctionType
ALU = mybir.AluOpType

NH = 12          # heads
DH = 64          # head dim
HD = 768         # model dim
FF = 3072        # ffn dim
W = 256          # one-sided window
L = 4            # layers
B = 2
S = 4096
EPS = 1e-12
N_CORES = 8
T_OWN = 1024     # tokens per core
T_EXT = 1536     # with halos
FT = 6           # model-dim 128-tiles
FFT = 24         # ffn-dim 128-tiles
NCH = 4          # local chunks of 256 queries
P = 128

# bias_all column layout: bq, bk, bo, b2, ls1, lb1, ls2, lb2 (6 each), b1 (24)
BC_BQ, BC_BK, BC_BO, BC_B2 = 0, 6, 12, 18
BC_LS1, BC_LB1, BC_LS2, BC_LB2, BC_B1 = 24, 30, 36, 42, 48


def _ln_t(nc, sb, ps, r_aps, out_aps, ones_r, ones_b, eps_sb, bias_sb,
          s_col, b_col):
    """LayerNorm over the partition (feature) axis of transposed tiles.

    r_aps: FT fp32r SBUF APs [128, 512] (input; clobbered).
    out_aps[ft] <- LN(r)*s + b.  s/b from bias_sb columns s_col/b_col.
    Stats PSUM comes from the shared pool `ps` tag "ops" (bufs=2 required).
    """
    ncols = 512
    sumx = ps.tile([1, ncols], F32, tag="ops", name="lnsum", bufs=2)
    sumsq = ps.tile([1, ncols], F32, tag="ops", name="lnsumsq", bufs=2)
    for ft in range(FT):
        sq = sb.tile([P, ncols], BF16, tag="lnsq", name="lnsq", bufs=1)
        nc.scalar.activation(sq[:], r_aps[ft], AF.Square)
        nc.tensor.matmul(sumx[0:1, :], ones_r[:, 0:1], r_aps[ft],
                         start=(ft == 0), stop=(ft == FT - 1))
        nc.tensor.matmul(sumsq[0:1, :], ones_b[:, 0:1], sq[:],
                         start=(ft == 0), stop=(ft == FT - 1))
    mu = sb.tile([1, ncols], F32, tag="lnmu", name="lnmu")
    nc.scalar.activation(mu[:], sumx[0:1, :], AF.Identity, scale=1.0 / HD)
    musq = sb.tile([1, ncols], F32, tag="lnmusq", name="lnmusq")
    nc.vector.tensor_tensor(musq[:], mu[:], mu[:], op=ALU.mult)
    var = sb.tile([1, ncols], F32, tag="lnvar", name="lnvar")
    nc.vector.scalar_tensor_tensor(out=var[:], in0=sumsq[0:1, :],
                                   scalar=1.0 / HD, in1=musq[:],
                                   op0=ALU.mult, op1=ALU.subtract)
    rstd = sb.tile([1, ncols], F32, tag="lnrstd", name="lnrstd")
    nc.scalar.activation(rstd[:], var[:], AF.Abs_reciprocal_sqrt,
                         bias=eps_sb[0:1, :])
    mu_b = sb.tile([P, ncols], F32, tag="lnmub", name="lnmub")
    nc.gpsimd.partition_broadcast(mu_b[:], mu[:], channels=P)
    rstd_b = sb.tile([P, ncols], F32, tag="lnrstdb", name="lnrstdb")
    nc.gpsimd.partition_broadcast(rstd_b[:], rstd[:], channels=P)
    for ft in range(FT):
        nc.vector.tensor_tensor(r_aps[ft], r_aps[ft], mu_b[:], op=ALU.subtract)
        nc.vector.tensor_tensor(r_aps[ft], r_aps[ft], rstd_b[:], op=ALU.mult)
        nc.scalar.activation(out_aps[ft], r_aps[ft], AF.Identity,
                             scale=bias_sb[:, s_col + ft:s_col + ft + 1],
                             bias=bias_sb[:, b_col + ft:b_col + ft + 1])


def build_nc(n_layers=L):
    nc = bacc.Bacc("TRN2", target_bir_lowering=False, debug=False,
                   num_devices=N_CORES)
    dt_ = nc.dram_tensor
    t = {}
    t["emb"] = dt_("emb_word", [32000, HD], F32, kind="ExternalInput").ap()
    t["ids"] = dt_("ids", [P, 12], I32, kind="ExternalInput").ap()
    t["pos"] = dt_("pos", [T_EXT, HD], F32, kind="ExternalInput").ap()
    t["eln_s"] = dt_("eln_s", [HD], F32, kind="ExternalInput").ap()
    t["eln_b"] = dt_("eln_b", [HD], F32, kind="ExternalInput").ap()
    t["wqk"] = dt_("wqk", [L, P, 12 * HD], BF16, kind="ExternalInput").ap()
    t["wv"] = dt_("wv", [L, P, FT * HD], BF16, kind="ExternalInput").ap()
    t["wo"] = dt_("wo", [L, P, FT * HD], BF16, kind="ExternalInput").ap()
    t["w1"] = dt_("w1", [L, 8, P, 3 * HD], BF16, kind="ExternalInput").ap()
    t["w2"] = dt_("w2", [L, FT, P, FFT * P], BF16, kind="ExternalInput").ap()
    t["bias"] = dt_("bias", [L, P, 72], F32, kind="ExternalInput").ap()
    t["sel"] = dt_("sel", [NH, FT * P], BF16, kind="ExternalInput").ap()
    t["ml"] = dt_("ml", [NCH, P, 512], BF16, kind="ExternalInput").ap()
    t["mr"] = dt_("mr", [NCH, P, 512], BF16, kind="ExternalInput").ap()
    t["halo_ids"] = dt_("halo_ids", [P, 12], I32, kind="ExternalInput").ap()
    t["out"] = dt_("out", [FT, P, T_OWN], F32, kind="ExternalOutput").ap()

    with tile.TileContext(nc) as tc:
        _build_body(nc, tc, n_layers, t)
    nc.compile()
    return nc


def _build_body(nc, tc, n_layers, t):
    from contextlib import ExitStack
    with ExitStack() as ctx:
        persist = ctx.enter_context(tc.tile_pool(name="persist", bufs=1))
        # residual stream, transposed, bf16, with halos: x[ft] = [128, T_EXT]
        x = [persist.tile([P, T_EXT], BF16, tag=f"x{ft}", name=f"x{ft}")
             for ft in range(FT)]
        ml_sb = [persist.tile([P, 512], BF16, tag=f"ml{c}", name=f"ml{c}")
                 for c in range(NCH)]
        mr_sb = [persist.tile([P, 512], BF16, tag=f"mr{c}", name=f"mr{c}")
                 for c in range(NCH)]
        for c in range(NCH):
            nc.sync.dma_start(ml_sb[c][:], t["ml"][c])
            nc.sync.dma_start(mr_sb[c][:], t["mr"][c])
        sel_sb = persist.tile([NH, FT * P], BF16, tag="sel", name="sel")
        nc.sync.dma_start(sel_sb[:], t["sel"][:])
        ones_f = persist.tile([P, 1], F32, tag="ones_f", name="ones_f")
        nc.vector.memset(ones_f[:], 1.0)
        ones_r = persist.tile([P, 1], F32R, tag="ones_r", name="ones_r")
        nc.scalar.activation(ones_r[:], ones_f[:], AF.Identity)
        ones_b = persist.tile([P, 1], BF16, tag="ones_b", name="ones_b")
        nc.scalar.activation(ones_b[:], ones_f[:], AF.Identity)
        from concourse.masks import make_identity
        ident = persist.tile([P, P], F32, tag="ident", name="ident")
        make_identity(nc, ident[:])
        hid_sb = persist.tile([P, 12], I32, tag="hid", name="hid")
        nc.sync.dma_start(hid_sb[:], t["halo_ids"][:])
        eps_sb = persist.tile([P, 1], F32, tag="eps", name="eps")
        nc.vector.memset(eps_sb[:], EPS)

        # ---- embedding + LN -> x^T (bf16, full ext incl. halos) ----
        with tc.tile_pool(name="emb_sb", bufs=1) as esb, \
             tc.tile_pool(name="emb_sb2", bufs=2) as esb2, \
             tc.tile_pool(name="emb_ps", bufs=2, space="PSUM") as eps_ps:
            ids_sb = esb.tile([P, 12], I32, tag="ids", name="ids")
            nc.sync.dma_start(ids_sb[:], t["ids"][:])
            s_bc = esb.tile([P, HD], F32, tag="sbc", name="sbc")
            nc.sync.dma_start(s_bc[:], bass.AP(
                tensor=t["eln_s"].tensor, offset=0, ap=[[0, P], [1, HD]]))
            b_bc = esb.tile([P, HD], F32, tag="bbc", name="bbc")
            nc.sync.dma_start(b_bc[:], bass.AP(
                tensor=t["eln_b"].tensor, offset=0, ap=[[0, P], [1, HD]]))
            e = [esb.tile([P, HD], F32, tag=f"e{tt}", name=f"e{tt}")
                 for tt in range(12)]
            for tt in range(12):
                nc.gpsimd.indirect_dma_start(
                    out=e[tt][:], out_offset=None, in_=t["emb"][:],
                    in_offset=bass.IndirectOffsetOnAxis(
                        ap=ids_sb[:, tt:tt + 1], axis=0))
                p_sb = esb2.tile([P, HD], F32, tag="pos", name="pos")
                nc.sync.dma_start(p_sb[:], t["pos"][tt * P:(tt + 1) * P, :])
                nc.vector.tensor_tensor(e[tt][:], e[tt][:], p_sb[:], op=ALU.add)
                stats = esb2.tile([P, 3, nc.vector.BN_STATS_DIM], F32,
                                  tag="bst", name="bst")
                er = e[tt][:].rearrange("p (g d) -> p g d", g=3)
                for g in range(3):
                    nc.vector.bn_stats(stats[:, g, :], er[:, g, :])
                mv = esb2.tile([P, nc.vector.BN_AGGR_DIM], F32, tag="bag",
                               name="bag")
                nc.vector.bn_aggr(mv[:], stats[:])
                sd = esb2.tile([P, 1], F32, tag="bsd", name="bsd")
                nc.scalar.activation(sd[:], mv[:, 1:2], AF.Sqrt, bias=eps_sb[:])
                rstd = esb2.tile([P, 1], F32, tag="brstd", name="brstd")
                nc.vector.reciprocal_approx_fast(rstd[:], sd[:])
                nc.vector.tensor_scalar(out=e[tt][:], in0=e[tt][:],
                                        scalar1=mv[:, 0:1], scalar2=rstd[:],
                                        op0=ALU.subtract, op1=ALU.mult)
                nc.vector.tensor_tensor(e[tt][:], e[tt][:], s_bc[:],
                                        op=ALU.mult)
                nc.vector.tensor_tensor(e[tt][:], e[tt][:], b_bc[:],
                                        op=ALU.add)
            for ft in range(FT):
                tr = eps_ps.tile([P, T_EXT], F32, tag="tr", name="tr")
                for tt in range(12):
                    nc.tensor.transpose(tr[:, tt * P:(tt + 1) * P],
                                        e[tt][:, ft * P:(ft + 1) * P],
                                        ident[:])
                nc.scalar.activation(x[ft][:], tr[:], AF.Identity)

        scatter_fn = None
        for l in range(n_layers):
            scatter_fn = _layer(nc, tc, t, l, x, ml_sb, mr_sb, ones_r, ones_b,
                                eps_sb, hid_sb, sel_sb, scatter_fn,
                                exchange=(l < n_layers - 1),
                                last=(l == n_layers - 1))


def _layer(nc, tc, t, l, x, ml_sb, mr_sb, ones_r, ones_b, eps_sb, hid_sb,
           sel_sb, consume_halo, exchange, last):
    from contextlib import ExitStack
    with ExitStack() as ctx:
        lsb = ctx.enter_context(tc.tile_pool(name=f"lsb{l}", bufs=1))
        lps = ctx.enter_context(tc.tile_pool(name=f"lps{l}", bufs=1,
                                             space="PSUM"))

        bias_sb = lsb.tile([P, 72], F32, tag="bias", name="bias")
        nc.sync.dma_start(bias_sb[:], t["bias"][l])
        wo_sb = lsb.tile([P, FT * HD], BF16, tag="wo", name="wo")
        nc.sync.dma_start(wo_sb[:], t["wo"][l])

        qT = [lsb.tile([P, T_OWN], BF16, tag=f"qT{i}", name=f"qT{i}")
              for i in range(FT)]
        kT = [lsb.tile([P, T_EXT], BF16, tag=f"kT{i}", name=f"kT{i}")
              for i in range(FT)]
        v = [lsb.tile([P, NH, 65], BF16, tag=f"v{i}", name=f"v{i}")
             for i in range(12)]
        o = [lsb.tile([P, T_OWN], BF16, tag=f"o{i}", name=f"o{i}")
             for i in range(FT)]
        r1 = [lsb.tile([P, T_OWN], F32R, tag=f"r1_{i}", name=f"r1_{i}")
              for i in range(FT)]
        den = lsb.tile([NH, T_OWN], F32, tag="den", name="den")

        # y (LN1 out) aliases kT's first T_OWN columns (kT dead by then)
        y = [kT[i][:, 0:T_OWN] for i in range(FT)]
        # r2 aliases r1 (r1 dead after LN1 consumed it)
        r2 = r1

        paw = tc.alloc_tile_pool(name=f"paw{l}", bufs=1)
        wqk = paw.tile([P, 12 * HD], BF16, tag="wqk", name="wqk")
        nc.sync.dma_start(wqk[:], t["wqk"][l])
        wv_sb = paw.tile([P, FT * HD], BF16, tag="wv", name="wv")
        nc.sync.dma_start(wv_sb[:], t["wv"][l])

        def proj_qT(mt, h2):
            ws = wqk[:, mt * HD:(mt + 1) * HD]
            ps = lps.tile([P, 512], F32, tag="pp", name="pq", bufs=3)
            for kt in range(FT):
                nc.tensor.matmul(
                    ps[:], ws[:, kt * P:(kt + 1) * P],
                    x[kt][:, W + h2 * 512:W + (h2 + 1) * 512],
                    start=(kt == 0), stop=(kt == FT - 1))
            nc.scalar.activation(qT[mt][:, h2 * 512:(h2 + 1) * 512], ps[:],
                                 AF.Identity,
                                 bias=bias_sb[:, BC_BQ + mt:BC_BQ + mt + 1])

        def proj_kT(mt, c0, n):
            ws = wqk[:, (6 + mt) * HD:(7 + mt) * HD]
            ps = lps.tile([P, 512], F32, tag="pp", name="pk", bufs=3)
            for kt in range(FT):
                nc.tensor.matmul(ps[:, 0:n], ws[:, kt * P:(kt + 1) * P],
                                 x[kt][:, c0:c0 + n],
                                 start=(kt == 0), stop=(kt == FT - 1))
            nc.scalar.activation(kT[mt][:, c0:c0 + n], ps[:, 0:n],
                                 AF.Identity,
                                 bias=bias_sb[:, BC_BK + mt:BC_BK + mt + 1])

        def proj_v(tt, hf):
            ps = lps.tile([P, 512], F32, tag="pp", name="pv", bufs=3)
            for kt in range(FT):
                nc.tensor.matmul(
                    ps[:, 0:384], x[kt][:, tt * P:(tt + 1) * P],
                    wv_sb[:, kt * HD + hf * 384:kt * HD + (hf + 1) * 384],
                    start=(kt == 0), stop=(kt == FT - 1))
            nc.scalar.activation(
                v[tt][:, hf * 6:(hf + 1) * 6, 0:64],
                ps[:, 0:384].rearrange("p (h d) -> p h d", h=6), AF.Identity)
            if hf == 1:
                nc.vector.memset(v[tt][:, :, 64:65], 1.0)

        def attn_head(c, h):
            ft, po = h // 2, (h % 2) * 64
            sps = lps.tile([P, 6 * W], F32, tag="sps", name="sps", bufs=1)
            for w in range(6):
                nc.tensor.matmul(
                    sps[:, w * W:(w + 1) * W],
                    kT[ft][po:po + 64, c * W + w * P:c * W + (w + 1) * P],
                    qT[ft][po:po + 64, c * W:(c + 1) * W],
                    start=True, stop=True)
            ex = lsb.tile([P, 6 * W], BF16, tag="ex", name="ex", bufs=2)
            nc.scalar.activation(ex[:], sps[:], AF.Exp)
            nc.vector.tensor_tensor(ex[:, 0:512], ex[:, 0:512], ml_sb[c][:],
                                    op=ALU.mult)
            nc.vector.tensor_tensor(ex[:, 1024:1536], ex[:, 1024:1536],
                                    mr_sb[c][:], op=ALU.mult)
            ops = lps.tile([P, W], F32, tag="ops", name="ops", bufs=2)
            for w in range(6):
                nc.tensor.matmul(ops[0:65, :], v[c * 2 + w][:, h, :],
                                 ex[:, w * W:(w + 1) * W],
                                 start=(w == 0), stop=(w == 5))
            sden = lsb.tile([1, W], F32, tag="sden", name="sden", bufs=2)
            nc.vector.tensor_copy(sden[:], ops[64:65, :])
            nc.gpsimd.dma_start(den[h:h + 1, c * W:(c + 1) * W], sden[:])
            nc.vector.tensor_copy(o[ft][po:po + 64, c * W:(c + 1) * W],
                                  ops[0:64, :])

        def attn_chunk(c, fillers):
            for h in range(NH):
                attn_head(c, h)
                if fillers:
                    fillers.pop(0)()
            normalize_chunk(c)

        def normalize_chunk(c):
            cs = slice(c * W, (c + 1) * W)
            rec = lsb.tile([NH, W], F32, tag="rec", name="rec", bufs=2)
            nc.vector.reciprocal(rec[:], den[0:NH, cs])
            recb = lsb.tile([NH, W], BF16, tag="recb", name="recb", bufs=2)
            nc.scalar.activation(recb[:], rec[:], AF.Identity)
            for ft in range(FT):
                bc = lps.tile([P, W], F32, tag="ops", name="bc", bufs=2)
                nc.tensor.matmul(bc[:], sel_sb[0:NH, ft * P:(ft + 1) * P],
                                 recb[:], start=True, stop=True)
                nc.vector.tensor_tensor(o[ft][:, cs], o[ft][:, cs], bc[:],
                                        op=ALU.mult)

        def oproj_mt(h2, mt):
            cs = slice(h2 * 512, (h2 + 1) * 512)
            ps = lps.tile([P, 512], F32, tag="pp", name="po", bufs=3)
            for kt in range(FT):
                nc.tensor.matmul(ps[:],
                                 wo_sb[:, mt * HD + kt * P:
                                       mt * HD + (kt + 1) * P],
                                 o[kt][:, cs],
                                 start=(kt == 0), stop=(kt == FT - 1))
            nc.vector.scalar_tensor_tensor(
                out=r1[mt][:, cs], in0=ps[:],
                scalar=bias_sb[:, BC_BO + mt:BC_BO + mt + 1],
                in1=x[mt][:, W + h2 * 512:W + (h2 + 1) * 512],
                op0=ALU.add, op1=ALU.add)

        # ================= emission =================
        # A (halo-independent): qT h0, kT own, v own tt2..7
        for mt in range(FT):
            proj_qT(mt, 0)
        for mt in range(FT):
            proj_kT(mt, W, 512)
            proj_kT(mt, W + 512, 512)
        for tt in range(2, 8):
            proj_v(tt, 0)
            proj_v(tt, 1)

        # B(c1) with fillers: qT h1 (6), v tt8,9 (4)
        f1 = [lambda mt=mt: proj_qT(mt, 1) for mt in range(FT)]
        f1 += [lambda tt=tt, hf=hf: proj_v(tt, hf)
               for tt in (8, 9) for hf in (0, 1)]
        attn_chunk(1, f1)

        # halo arrives: scatter previous layer's collective into x halo cols
        if consume_halo is not None:
            consume_halo()

        # B(c2) with fillers: kT halo-left (6), v tt0,1 (4)
        f2 = [lambda mt=mt: proj_kT(mt, 0, W) for mt in range(FT)]
        f2 += [lambda tt=tt, hf=hf: proj_v(tt, hf)
               for tt in (0, 1) for hf in (0, 1)]
        attn_chunk(2, f2)

        # B(c0) with fillers: kT halo-right (6), v tt10,11 (4)
        f0 = [lambda mt=mt: proj_kT(mt, T_OWN + W, W) for mt in range(FT)]
        f0 += [lambda tt=tt, hf=hf: proj_v(tt, hf)
               for tt in (10, 11) for hf in (0, 1)]
        attn_chunk(0, f0)
        paw.release()

        # B(c3) interleaved with O-proj(h0) per-mt fillers
        f3 = [lambda mt=mt: oproj_mt(0, mt) for mt in range(FT)]
        attn_chunk(3, f3)

        # LN1 h0; then O-proj h1 + LN1 h1 (PE does C(h1) while LN1(h0)
        # vector/scalar chain runs)
        _ln_t(nc, lsb, lps, [r1[ft][:, 0:512] for ft in range(FT)],
              [y[ft][:, 0:512] for ft in range(FT)],
              ones_r, ones_b, eps_sb, bias_sb, BC_LS1, BC_LB1)
        for mt in range(FT):
            oproj_mt(1, mt)
        _ln_t(nc, lsb, lps, [r1[ft][:, 512:1024] for ft in range(FT)],
              [y[ft][:, 512:1024] for ft in range(FT)],
              ones_r, ones_b, eps_sb, bias_sb, BC_LS1, BC_LB1)

        # FFN pools (opened after paw released)
        pd = ctx.enter_context(tc.tile_pool(name=f"pd{l}", bufs=1))

        def ffn_pass1(h2, f_list):
            cs = slice(h2 * 512, (h2 + 1) * 512)
            for ms in range(FFT):
                if ms % 3 == 0:
                    w1c = pd.tile([P, 3 * HD], BF16, tag="w1c", name="w1c",
                                  bufs=2)
                    nc.sync.dma_start(w1c[:], t["w1"][l, ms // 3])
                    ffn_pass1.w1c = w1c
                ws = ffn_pass1.w1c[:, (ms % 3) * HD:(ms % 3 + 1) * HD]
                fp = lps.tile([P, 512], F32, tag="pp", name="fp", bufs=3)
                for kt in range(FT):
                    nc.tensor.matmul(fp[:], ws[:, kt * P:(kt + 1) * P],
                                     y[kt][:, cs],
                                     start=(kt == 0), stop=(kt == FT - 1))
                fsb = pd.tile([P, 512], BF16, tag="fc", name="fsb", bufs=24)
                nc.scalar.activation(fsb[:], fp[:], AF.Gelu,
                                     bias=bias_sb[:, BC_B1 + ms:BC_B1 + ms + 1])
                f_list.append(fsb)

        def ffn_pass2(h2, f_list):
            cs = slice(h2 * 512, (h2 + 1) * 512)
            for mt in range(FT):
                w2p = pd.tile([P, FFT * P], BF16, tag="w2p", name="w2p",
                              bufs=2)
                nc.sync.dma_start(w2p[:], t["w2"][l, mt])
                zp = lps.tile([P, 512], F32, tag="ops", name="zp", bufs=2)
                for ms in range(FFT):
                    nc.tensor.matmul(zp[:], w2p[:, ms * P:(ms + 1) * P],
                                     f_list[ms][:],
                                     start=(ms == 0), stop=(ms == FFT - 1))
                nc.vector.scalar_tensor_tensor(
                    out=r2[mt][:, cs], in0=zp[:],
                    scalar=bias_sb[:, BC_B2 + mt:BC_B2 + mt + 1],
                    in1=y[mt][:, cs], op0=ALU.add, op1=ALU.add)

        def ln2_half(h2):
            csx = slice(W + h2 * 512, W + (h2 + 1) * 512)
            cs = slice(h2 * 512, (h2 + 1) * 512)
            if last:
                xo = pd.tile([P, FT, 512], F32, tag="xout", name="xo", bufs=1)
                _ln_t(nc, lsb, lps, [r2[ft][:, cs] for ft in range(FT)],
                      [xo[:, ft, :] for ft in range(FT)],
                      ones_r, ones_b, eps_sb, bias_sb, BC_LS2, BC_LB2)
                for ft in range(FT):
                    nc.gpsimd.dma_start(
                        t["out"][ft, :, h2 * 512:(h2 + 1) * 512],
                        xo[:, ft, :])
            else:
                _ln_t(nc, lsb, lps, [r2[ft][:, cs] for ft in range(FT)],
                      [x[ft][:, csx] for ft in range(FT)],
                      ones_r, ones_b, eps_sb, bias_sb, BC_LS2, BC_LB2)

        # FFN both halves back-to-back (one dense PE region, one Gelu-table
        # era); LN2 halves afterwards
        f_h0 = []
        ffn_pass1(0, f_h0)
        ffn_pass2(0, f_h0)
        f_h1 = []
        ffn_pass1(1, f_h1)
        ffn_pass2(1, f_h1)
        if exchange:
            edram = ctx.enter_context(
                tc.tile_pool(name=f"pe{l}", bufs=1, space="DRAM"))
            b_in = edram.tile([2, FT, P, W], BF16, tag="bin", name="bin")
            b_out = edram.tile([4 * 2 * FT * P, W], BF16, tag="bout",
                               name="bout")
        ln2_half(0)
        if exchange:
            for ft in range(FT):
                nc.gpsimd.dma_start(b_in[0, ft], x[ft][:, W:2 * W])
        ln2_half(1)

        if not exchange:
            return None

        for ft in range(FT):
            nc.gpsimd.dma_start(b_in[1, ft], x[ft][:, T_OWN:T_OWN + W])
        nc.gpsimd.collective_compute(
            "AllGather", ALU.bypass,
            replica_groups=[[0, 1, 2, 3], [4, 5, 6, 7]],
            ins=[b_in[:].opt()], outs=[b_out[:].opt()])

        def scatter():
            for side in range(2):
                for ft in range(FT):
                    dst = (x[ft][:, 0:W] if side == 0
                           else x[ft][:, T_OWN + W:T_EXT])
                    nc.gpsimd.indirect_dma_start(
                        out=dst, out_offset=None, in_=b_out[:],
                        in_offset=bass.IndirectOffsetOnAxis(
                            ap=hid_sb[:, side * FT + ft:side * FT + ft + 1],
                            axis=0))
        return scatter


# ---------------- host side ----------------

def _blocked(w, n_k, n_m):
    """[n_k*128, n_m*128] -> [n_m, 128, n_k, 128] (lhsT strips by out-tile)."""
    return np.ascontiguousarray(
        w.reshape(n_k, P, n_m, P).transpose(2, 1, 0, 3))


def _bias_lay(b, n):
    return np.ascontiguousarray(b.reshape(n, P).T)


def prepare(inputs):
    """Build per-core in_maps from full inputs."""
    bf = ml_dtypes.bfloat16
    ids_full = np.asarray(inputs["input_ids"]).astype(np.int32)
    am = np.asarray(inputs["attention_mask"]).astype(np.int32)
    emb_word = np.asarray(inputs["emb_word"], dtype=np.float32)
    emb_pos = np.asarray(inputs["emb_pos"], dtype=np.float32)
    Wq = np.asarray(inputs["Wq"], np.float32) / np.sqrt(DH)
    bq = np.asarray(inputs["bq"], np.float32) / np.sqrt(DH)
    Wk = np.asarray(inputs["Wk"], np.float32)
    bk = np.asarray(inputs["bk"], np.float32)
    Wv = np.asarray(inputs["Wv"], np.float32)
    bv = np.asarray(inputs["bv"], np.float32)
    Wo = np.asarray(inputs["Wo"], np.float32)
    bo = np.asarray(inputs["bo"], np.float32)
    W1 = np.asarray(inputs["W1"], np.float32)
    b1 = np.asarray(inputs["b1"], np.float32)
    W2 = np.asarray(inputs["W2"], np.float32)
    b2 = np.asarray(inputs["b2"], np.float32)
    assert np.all(am == 1), "general attention_mask needs mid-tile masks too"

    wqk = np.empty((L, P, 12 * HD), np.float32)
    wv_flat = np.empty((L, P, FT * HD), np.float32)
    wo_flat = np.empty((L, P, FT * HD), np.float32)
    w1c = np.empty((L, 8, P, 3 * HD), np.float32)
    w2p = np.empty((L, FT, P, FFT * P), np.float32)
    for i in range(L):
        bq_s = _blocked(Wq[i], FT, FT)   # [mt, P, kt*P]
        bk_s = _blocked(Wk[i], FT, FT)
        bo_s = _blocked(Wo[i], FT, FT)
        for mt in range(FT):
            wqk[i, :, mt * HD:(mt + 1) * HD] = bq_s[mt].reshape(P, HD)
            wqk[i, :, (6 + mt) * HD:(7 + mt) * HD] = bk_s[mt].reshape(P, HD)
            wo_flat[i, :, mt * HD:(mt + 1) * HD] = bo_s[mt].reshape(P, HD)
        wv_flat[i] = Wv[i].reshape(FT, P, HD).transpose(1, 0, 2).reshape(
            P, FT * HD)
        b1_s = _blocked(W1[i], FT, FFT)  # [FFT, P, HD]
        w1c[i] = b1_s.reshape(8, 3, P, HD).transpose(0, 2, 1, 3).reshape(
            8, P, 3 * HD)
        w2p[i] = W2[i].reshape(FFT, P, FT, P).transpose(2, 1, 0, 3).reshape(
            FT, P, FFT * P)

    bias_all = np.zeros((L, P, 72), np.float32)
    for i in range(L):
        bias_all[i, :, BC_BQ:BC_BQ + 6] = _bias_lay(bq[i], FT)
        bias_all[i, :, BC_BK:BC_BK + 6] = _bias_lay(bk[i], FT)
        bias_all[i, :, BC_BO:BC_BO + 6] = _bias_lay(bv[i] @ Wo[i] + bo[i], FT)
        bias_all[i, :, BC_B2:BC_B2 + 6] = _bias_lay(b2[i], FT)
        bias_all[i, :, BC_LS1:BC_LS1 + 6] = _bias_lay(
            np.asarray(inputs["ln1_s"], np.float32)[i], FT)
        bias_all[i, :, BC_LB1:BC_LB1 + 6] = _bias_lay(
            np.asarray(inputs["ln1_b"], np.float32)[i], FT)
        bias_all[i, :, BC_LS2:BC_LS2 + 6] = _bias_lay(
            np.asarray(inputs["ln2_s"], np.float32)[i], FT)
        bias_all[i, :, BC_LB2:BC_LB2 + 6] = _bias_lay(
            np.asarray(inputs["ln2_b"], np.float32)[i], FT)
        bias_all[i, :, BC_B1:BC_B1 + 24] = _bias_lay(b1[i], FFT)

    sel = np.zeros((NH, FT * P), np.float32)
    for ft in range(FT):
        for p in range(P):
            sel[2 * ft + (p >= 64), ft * P + p] = 1.0

    shared = {
        "emb_word": emb_word,
        "eln_s": np.asarray(inputs["emb_ln_s"], np.float32),
        "eln_b": np.asarray(inputs["emb_ln_b"], np.float32),
        "wqk": wqk.astype(bf),
        "wv": wv_flat.astype(bf),
        "wo": wo_flat.astype(bf),
        "w1": w1c.astype(bf),
        "w2": w2p.astype(bf),
        "bias": bias_all,
        "sel": sel.astype(bf),
    }

    in_maps = []
    i_idx = np.arange(W)
    for core in range(N_CORES):
        b, sb = core // 4, core % 4
        s0 = sb * T_OWN
        ext_pos = np.clip(np.arange(s0 - W, s0 + T_OWN + W), 0, S - 1)
        m = dict(shared)
        m["ids"] = np.ascontiguousarray(
            ids_full[b, ext_pos].reshape(12, P).T)
        m["pos"] = np.ascontiguousarray(emb_pos[ext_pos])
        # masks: global chunk gc, window key j in [0,768), query i in [0,256):
        #   key_abs = gc*W - W + j ; allowed = |j - W - i| <= W
        #             & 0 <= key_abs < S & attention_mask[b, key_abs]
        mlm = np.zeros((NCH, P, 512), np.float32)
        mrm = np.zeros((NCH, P, 512), np.float32)
        for c in range(NCH):
            gc = sb * NCH + c
            for kt2 in range(2):
                for mm_, j0 in ((mlm, 0), (mrm, 512)):
                    j = j0 + kt2 * P + np.arange(P)[:, None]
                    key_abs = gc * W - W + j
                    ok = (np.abs(j - W - i_idx[None, :]) <= W)
                    ok &= (key_abs >= 0) & (key_abs < S)
                    ok &= am[b, np.clip(key_abs, 0, S - 1)] > 0
                    mm_[c, :, kt2 * W:(kt2 + 1) * W] = ok
        m["ml"] = mlm.astype(bf)
        m["mr"] = mrm.astype(bf)
        # halo row ids into the gathered [4, 2, FT, 128, W] row table
        hid = np.zeros((2, FT, P), np.int64)
        for side in range(2):
            nb = sb - 1 if side == 0 else sb + 1
            if 0 <= nb <= 3:
                osd = 1 - side  # left halo <- neighbor's right block
                for ft in range(FT):
                    hid[side, ft] = ((nb * 2 + osd) * FT + ft) * P \
                        + np.arange(P)
            else:
                for ft in range(FT):
                    hid[side, ft] = ((sb * 2 + side) * FT + ft) * P \
                        + np.arange(P)
        m["halo_ids"] = np.ascontiguousarray(
            hid.reshape(12, P).T.astype(np.int32))
        in_maps.append(m)
    return in_maps


_NC_CACHE = {}


def get_nc(n_layers=L):
    if n_layers not in _NC_CACHE:
        _NC_CACHE[n_layers] = build_nc(n_layers)
    return _NC_CACHE[n_layers]


def run(inputs, n_layers=L, trace=False):
    nc = get_nc(n_layers)
    in_maps = prepare(inputs)
    res = bass_utils.run_bass_kernel_spmd(
        nc, in_maps, core_ids=list(range(N_CORES)), trace=trace)
    outs = np.empty((B, S, HD), np.float32)
    for core in range(N_CORES):
        b, sb = core // 4, core % 4
        ot = res.results[core]["out"]  # [FT, 128, T_OWN]
        outs[b, sb * T_OWN:(sb + 1) * T_OWN] = ot.reshape(HD, T_OWN).T
    return outs, res


def kernel(**inputs) -> np.ndarray:
    out, _ = run(inputs)
    return out
